# revision 1
# baseline (speedup 1.0000x reference)
"""Trainium2 Bass kernel for CenterGeoAttention (N=65536, D=1024, H=16).

Strategy (row-shard N across 8 cores, activations kept feature-major):

Host algebra reduces the attention almost entirely:
  - q = LN(h[c]) @ Wq is input-only -> fold into Wkp = (Wk @ Qblockdiag) * gamma_a
    (1024x16), so logits need no K projection matmul.
  - LN is folded into rank-1 corrections around raw-h matmuls (means/rstd are
    per-row column scalings that commute with the feature-contraction).
  - The weighted V sum never materializes V: G = (p*r)^T @ h (16x1024 per core),
    AllReduce-add [G | PRM | S], then out_center = blockdiag(G_hat @ Wv),
    h_c_new = h[c] + 0.5 * Wo^T @ out_center.
  - h_cat @ W = h @ W_top + rank-1(h_c_new @ W_bot) splits the 2D-wide MLP/gate
    matmuls in half.
Device per core: 3 big fp32r matmuls (h@W1t, h@Wgt, silu@W2) of 8192x1024x1024
plus the cheap attention pass and one 66KB AllReduce.
"""

import os
import ml_dtypes
import numpy as np

import concourse.bass as bass
import concourse.bacc as bacc
import concourse.tile as tile
import concourse.mybir as mybir
from concourse.bass_utils import run_bass_kernel_spmd

F32 = mybir.dt.float32
F32R = mybir.dt.float32r
BF16 = mybir.dt.bfloat16
AF = mybir.ActivationFunctionType
OP = mybir.AluOpType
AX = mybir.AxisListType

NCORES = 8
N, D, H, HD, BIAS = 65536, 1024, 16, 64, 128
NS = N // NCORES            # 8192 rows per core
CH = 512                    # row-chunk
NCH = NS // CH              # 16 chunks
KT = D // 128               # 8 feature tiles
EPS = 1e-5
RES = 0.5

_CACHE = {}
LAST_RESULTS = None  # BassKernelResults from the most recent run (for test.py)


def _build(ncores=NCORES, variant="full", nch=NCH, stage=99):
    nc = bacc.Bacc("TRN2", target_bir_lowering=False, debug=False,
                   num_devices=ncores)

    def din(name, shape, dt=F32R):
        return nc.dram_tensor(name, list(shape), dt, kind="ExternalInput").ap()

    # per-core tensors
    hT = din("hT", (D, NS))               # h_shard^T
    hN = din("hN", (NS, D), BF16)         # h_shard natural (bf16)
    bT = din("bT", (BIAS, NS), BF16)      # bias_feat^T shard (bf16)
    # shared weights
    Wkp = din("Wkp", (D, H))
    Wb = din("Wb", (BIAS, H), BF16)
    W1t = din("W1t", (D, D))
    Wgt = din("Wgt", (D, D))
    W2h = din("W2h", (D, D))
    Wv = din("Wv", (D, D), BF16)
    Wo = din("Wo", (D, D), BF16)
    W1b = din("W1b", (D, D), BF16)
    Wgb = din("Wgb", (D, D), BF16)
    # small constants
    idn = din("idn", (128, 128), F32)
    ones128 = din("ones128", (128, 1), F32R)
    ncg = din("ncg", (H, 1), F32)         # -cg per head
    cbv = din("cbv", (H, 1), F32)         # cb per head (exp bias)
    gb16 = din("gb16", (H, D), F32)       # gamma_a broadcast rows
    bb16 = din("bb16", (H, D), F32)       # beta_a broadcast rows
    hcv = din("hcv", (128, KT), F32)      # h[c] as [p, m]
    b1v = din("b1v", (128, KT), F32)
    bgv = din("bgv", (128, KT), F32)
    b2v = din("b2v", (128, KT), F32)      # 0.5*b2
    epsv = din("epsv", (1, 1), F32)

    outT = nc.dram_tensor("outT", [D, NS], F32, kind="ExternalOutput").ap()
    outC = nc.dram_tensor("outC", [128, KT], F32, kind="ExternalOutput").ap()

    with tile.TileContext(nc) as tc:
        with (
            tc.tile_pool(name="persist", bufs=1) as pp,
            tc.tile_pool(name="dram", bufs=1, space="DRAM") as dram,
        ):
            # ---- long-lived small tiles ----
            idn_s = pp.tile([128, 128], F32, tag="idn")
            nc.sync.dma_start(out=idn_s[:], in_=idn[:])
            ones_s = pp.tile([128, 1], F32R, tag="ones128")
            nc.sync.dma_start(out=ones_s[:], in_=ones128[:])
            ncg_s = pp.tile([H, 1], F32, tag="ncg")
            nc.sync.dma_start(out=ncg_s[:], in_=ncg[:])
            cbv_s = pp.tile([H, 1], F32, tag="cbv")
            nc.sync.dma_start(out=cbv_s[:], in_=cbv[:])
            hcv_s = pp.tile([128, KT], F32, tag="hcv")
            nc.sync.dma_start(out=hcv_s[:], in_=hcv[:])
            b1v_s = pp.tile([128, KT], F32, tag="b1v")
            nc.sync.dma_start(out=b1v_s[:], in_=b1v[:])
            bgv_s = pp.tile([128, KT], F32, tag="bgv")
            nc.sync.dma_start(out=bgv_s[:], in_=bgv[:])
            b2v_s = pp.tile([128, KT], F32, tag="b2v")
            nc.sync.dma_start(out=b2v_s[:], in_=b2v[:])
            Wkp_s = pp.tile([128, KT * H], F32R, tag="Wkp")
            for k in range(KT):
                nc.sync.dma_start(out=Wkp_s[:, k * H:(k + 1) * H],
                                  in_=Wkp[k * 128:(k + 1) * 128, :])
            Wb_s = pp.tile([BIAS, H], BF16, tag="Wb")
            nc.sync.dma_start(out=Wb_s[:], in_=Wb[:])
            epsv_s = pp.tile([1, 1], F32, tag="epsv")
            nc.sync.dma_start(out=epsv_s[:], in_=epsv[:])

            Gacc = pp.tile([H, D], F32, tag="Gacc")
            sCols = pp.tile([H, NCH], F32, tag="sCols")
            prmCols = pp.tile([H, NCH], F32, tag="prmCols")
            hcn_sb = pp.tile([128, KT], F32, tag="hcn")
            g0_s = pp.tile([128, KT], F32, tag="g0")
            a0_s = pp.tile([128, KT], F32, tag="a0")

            # resident pass-2 weights: loaded during pass 1
            wres_cm = tc.tile_pool(name="p2w", bufs=1)
            wres = wres_cm.__enter__()
            W1t_s = wres.tile([128, KT * D], F32R, tag="W1t")
            Wgt_s = wres.tile([128, KT * D], F32R, tag="Wgt")

            # =========================== PASS 1 ===========================
            psG_cm = tc.tile_pool(name="p1psG", bufs=1, space="PSUM")
            psG = psG_cm.__enter__()
            G = psG.tile([H, D], F32, tag="G")
            with (
                tc.tile_pool(name="p1sb", bufs=1) as sb1,
                tc.tile_pool(name="p1sb2", bufs=2) as sb2,
                tc.tile_pool(name="p1ps", bufs=1, space="PSUM") as ps1,
            ):
                for c in range(nch):
                    c0 = c * CH
                    if c == 2:
                        for k in range(KT):
                            nc.sync.dma_start(
                                out=W1t_s[:, k * D:(k + 1) * D],
                                in_=W1t[k * 128:(k + 1) * 128, :])
                            nc.sync.dma_start(
                                out=Wgt_s[:, k * D:(k + 1) * D],
                                in_=Wgt[k * 128:(k + 1) * 128, :])
                    hTc = sb2.tile([128, KT * CH], F32R, tag="hTc")
                    for k in range(KT):
                        nc.sync.dma_start(
                            out=hTc[:, k * CH:(k + 1) * CH],
                            in_=hT[k * 128:(k + 1) * 128, c0:c0 + CH])
                    hNc = sb2.tile([128, 4 * D], BF16, tag="hNc")
                    for j in range(4):
                        nc.sync.dma_start(
                            out=hNc[:, j * D:(j + 1) * D],
                            in_=hN[c0 + j * 128:c0 + (j + 1) * 128, :])
                    bTc = sb2.tile([BIAS, CH], BF16, tag="bTc")
                    nc.sync.dma_start(out=bTc[:], in_=bT[:, c0:c0 + CH])

                    if stage == 0:
                        ot0 = sb1.tile([128, CH], F32, tag="ot0")
                        nc.vector.tensor_copy(ot0[:], hTc[:, 0:CH].bitcast(F32))
                        nc.sync.dma_start(out=outT[0:128, c0:c0 + CH], in_=ot0[:])
                        continue
                    # row stats: sum(h), sum(h^2) via ones-matmuls
                    sq = sb1.tile([128, KT * CH], F32R, tag="sq")
                    nc.vector.tensor_mul(sq[:], hTc[:], hTc[:])
                    stats_m = ps1.tile([1, CH], F32, tag="stats_m")
                    for k in range(KT):
                        nc.tensor.matmul(stats_m[:], ones_s[:],
                                         hTc[:, k * CH:(k + 1) * CH],
                                         start=(k == 0), stop=(k == KT - 1))
                    stats_q = ps1.tile([1, CH], F32, tag="stats_q")
                    for k in range(KT):
                        nc.tensor.matmul(stats_q[:], ones_s[:],
                                         sq[:, k * CH:(k + 1) * CH],
                                         start=(k == 0), stop=(k == KT - 1))
                    tm = sb2.tile([1, CH], F32, tag="tm")
                    nc.vector.tensor_scalar_mul(tm[:], stats_m[:], 1.0 / D)
                    msq = sb2.tile([1, CH], F32, tag="msq")
                    nc.scalar.square(msq[:], tm[:])
                    var = sb2.tile([1, CH], F32, tag="var")
                    nc.vector.scalar_tensor_tensor(
                        var[:], stats_q[:], 1.0 / D, msq[:],
                        op0=OP.mult, op1=OP.subtract)
                    sd = sb2.tile([1, CH], F32, tag="sd")
                    nc.scalar.activation(sd[:], var[:], AF.Sqrt, bias=epsv_s[:, 0:1])
                    r_t = sb2.tile([1, CH], F32, tag="rt")
                    nc.vector.reciprocal(r_t[:], sd[:])
                    mr_t = sb2.tile([1, CH], F32, tag="mrt")
                    nc.vector.tensor_mul(mr_t[:], tm[:], r_t[:])

                    if stage == 1:
                        ot1 = sb1.tile([1, 2 * CH], F32, tag="ot1")
                        nc.vector.tensor_copy(ot1[:, 0:CH], r_t[:])
                        nc.vector.tensor_copy(ot1[:, CH:2 * CH], mr_t[:])
                        nc.sync.dma_start(out=outT[0:1, c0:c0 + 2 * CH], in_=ot1[:])
                        continue
                    # broadcast r and m*r to 16 partitions
                    rb16 = sb2.tile([H, CH], F32, tag="rb16")
                    nc.gpsimd.partition_broadcast(rb16[:], r_t[:])
                    mrb16 = sb2.tile([H, CH], F32, tag="mrb16")
                    nc.gpsimd.partition_broadcast(mrb16[:], mr_t[:])

                    L = ps1.tile([H, CH], F32, tag="L")
                    for k in range(KT):
                        nc.tensor.matmul(L[:], Wkp_s[:, k * H:(k + 1) * H],
                                         hTc[:, k * CH:(k + 1) * CH],
                                         start=(k == 0), stop=(k == KT - 1))
                    L2 = ps1.tile([H, CH], F32, tag="L2")
                    nc.tensor.matmul(L2[:], Wb_s[:], bTc[:],
                                     start=True, stop=True)
                    t3 = sb1.tile([H, CH], F32, tag="t3")
                    nc.vector.tensor_mul(t3[:], L[:], rb16[:])
                    t4 = sb2.tile([H, CH], F32, tag="t4")
                    nc.vector.scalar_tensor_tensor(
                        t4[:], mrb16[:], ncg_s[:, 0:1], t3[:],
                        op0=OP.mult, op1=OP.add)
                    t5 = sb2.tile([H, CH], F32, tag="t5")
                    nc.vector.tensor_add(t5[:], t4[:], L2[:])
                    if stage == 2:
                        nc.sync.dma_start(out=outT[0:H, c0:c0 + CH], in_=t5[:])
                        continue
                    pT = sb2.tile([H, CH], F32, tag="pT")
                    if stage == 30:
                        nc.scalar.activation(pT[:], t5[:], AF.Exp,
                                             bias=cbv_s[:, 0:1])
                        nc.sync.dma_start(out=outT[0:H, c0:c0 + CH], in_=pT[:])
                        continue
                    nc.scalar.activation(pT[:], t5[:], AF.Exp,
                                         bias=cbv_s[:, 0:1],
                                         accum_out=sCols[:, c:c + 1])
                    if stage == 31:
                        nc.sync.dma_start(out=outT[0:H, c0:c0 + CH], in_=pT[:])
                        continue
                    prT = sb2.tile([H, CH], F32, tag="prT")
                    nc.vector.tensor_mul(prT[:], pT[:], rb16[:])
                    prm_scr = sb1.tile([H, CH], F32, tag="prmscr")
                    nc.vector.tensor_mul(prm_scr[:], pT[:], mrb16[:])
                    nc.vector.reduce_sum(prmCols[:, c:c + 1], prm_scr[:],
                                         axis=AX.X)
                    if stage == 32:
                        nc.sync.dma_start(out=outT[0:H, c0:c0 + CH], in_=prT[:])
                        continue

                    if stage == 3:
                        nc.sync.dma_start(out=outT[0:H, c0:c0 + CH], in_=pT[:])
                        continue
                    # transpose p*r to row-major and accumulate G
                    tp = ps1.tile([128, 4 * H], F32, tag="tp")
                    for j in range(4):
                        nc.tensor.transpose(
                            tp[:, j * H:(j + 1) * H],
                            prT[:, j * 128:(j + 1) * 128],
                            idn_s[0:16, 0:16])
                    pr_nat = sb2.tile([128, 4 * H], BF16, tag="prnat")
                    nc.vector.tensor_copy(pr_nat[:], tp[:])
                    for half in range(2):
                        for j in range(4):
                            nc.tensor.matmul(
                                G[:, half * CH:(half + 1) * CH],
                                pr_nat[:, j * H:(j + 1) * H],
                                hNc[:, j * D + half * CH:j * D + (half + 1) * CH],
                                start=(c == 0 and j == 0),
                                stop=(c == nch - 1 and j == 3))
                nc.vector.tensor_copy(Gacc[:], G[:])
                if variant == "p1" and stage >= 4:
                    nc.sync.dma_start(out=outT[0:H, 0:D], in_=Gacc[:])
                    nc.sync.dma_start(out=outT[H:2 * H, 0:NCH], in_=sCols[:])
                    nc.sync.dma_start(out=outT[2 * H:3 * H, 0:NCH], in_=prmCols[:])

            if variant != "p1":
                psG_cm.__exit__(None, None, None)
            # ---- local partials -> AllReduce ----
                S16 = pp.tile([H, 1], F32, tag="S16")
                nc.vector.reduce_sum(S16[:], sCols[:], axis=AX.X)
                PRM16 = pp.tile([H, 1], F32, tag="PRM16")
                nc.vector.reduce_sum(PRM16[:], prmCols[:], axis=AX.X)

                arin = dram.tile([H, D + 2], F32, tag="arin")
                arout = dram.tile([H, D + 2], F32, tag="arout")
                nc.sync.dma_start(out=arin[:, 0:D], in_=Gacc[:])
                nc.sync.dma_start(out=arin[:, D:D + 1], in_=PRM16[:])
                nc.sync.dma_start(out=arin[:, D + 1:D + 2], in_=S16[:])
                if variant == "nocc":
                    nc.sync.dma_start(out=arout[:], in_=arin[:])
                else:
                    nc.gpsimd.collective_compute(
                        "AllReduce", OP.add,
                        replica_groups=[list(range(ncores))],
                        ins=[arin.opt()], outs=[arout.opt()])
                # ---- G corrections + normalize ----
                with (
                    tc.tile_pool(name="wstream", bufs=2) as ws,
                    tc.tile_pool(name="postsb", bufs=1) as psb,
                    tc.tile_pool(name="postps", bufs=1, space="PSUM") as ps2,
                ):
                    gb16_s = psb.tile([H, D], F32, tag="gb16")
                    nc.sync.dma_start(out=gb16_s[:], in_=gb16[:])
                    bb16_s = psb.tile([H, D], F32, tag="bb16")
                    nc.sync.dma_start(out=bb16_s[:], in_=bb16[:])
                    Gar = psb.tile([H, D], F32, tag="Gar")
                    nc.sync.dma_start(out=Gar[:], in_=arout[:, 0:D])
                    PSar = psb.tile([H, 2], F32, tag="PSar")
                    nc.sync.dma_start(out=PSar[:], in_=arout[:, D:D + 2])
                    Gn = psb.tile([H, D], F32, tag="Gn")
                    nc.vector.tensor_scalar_sub(Gn[:], Gar[:], PSar[:, 0:1])
                    nc.vector.tensor_mul(Gn[:], Gn[:], gb16_s[:])
                    nc.vector.scalar_tensor_tensor(
                        Gn[:], bb16_s[:], PSar[:, 1:2], Gn[:],
                        op0=OP.mult, op1=OP.add)
                    sr = psb.tile([H, 1], F32, tag="sr")
                    nc.vector.reciprocal(sr[:], PSar[:, 1:2])
                    nc.vector.tensor_scalar_mul(Gn[:], Gn[:], sr[:, 0:1])

                    tpg = ps2.tile([128, KT * H], F32, tag="tpg")
                    for m in range(KT):
                        nc.tensor.transpose(
                            tpg[:, m * H:(m + 1) * H],
                            Gn[:, m * 128:(m + 1) * 128],
                            idn_s[0:16, 0:16])
                    GnT = pp.tile([128, KT * H], BF16, tag="GnT")
                    nc.vector.tensor_copy(GnT[:], tpg[:])

                    Wv_s = ws.tile([128, KT * D], BF16, tag="wstream")
                    for k in range(KT):
                        nc.sync.dma_start(out=Wv_s[:, k * D:(k + 1) * D],
                                          in_=Wv[k * 128:(k + 1) * 128, :])
                    OCp = ps2.tile([128, KT * H], F32, tag="OCp")
                    for m in range(KT):
                        for k in range(KT):
                            nc.tensor.matmul(
                                OCp[:, m * H:(m + 1) * H],
                                Wv_s[:, k * D + m * 128:k * D + (m + 1) * 128],
                                GnT[:, k * H:(k + 1) * H],
                                start=(k == 0), stop=(k == KT - 1))
                    ocv = pp.tile([128, KT], BF16, tag="ocv")
                    for m in range(KT):
                        nc.vector.tensor_copy(
                            ocv[0:64, m:m + 1],
                            OCp[0:64, m * H + 2 * m:m * H + 2 * m + 1])
                        nc.vector.tensor_copy(
                            ocv[64:128, m:m + 1],
                            OCp[64:128, m * H + 2 * m + 1:m * H + 2 * m + 2])

                    Wo_s = ws.tile([128, KT * D], BF16, tag="wstream")
                    for k in range(KT):
                        nc.sync.dma_start(out=Wo_s[:, k * D:(k + 1) * D],
                                          in_=Wo[k * 128:(k + 1) * 128, :])
                    hcp = ps2.tile([128, KT], F32, tag="hcp")
                    for m in range(KT):
                        for k in range(KT):
                            nc.tensor.matmul(
                                hcp[:, m:m + 1],
                                Wo_s[:, k * D + m * 128:k * D + (m + 1) * 128],
                                ocv[:, k:k + 1],
                                start=(k == 0), stop=(k == KT - 1))
                    nc.vector.scalar_tensor_tensor(
                        hcn_sb[:], hcp[:], RES, hcv_s[:],
                        op0=OP.mult, op1=OP.add)
                    nc.sync.dma_start(out=outC[:], in_=hcn_sb[:])
                    hcn_bf = pp.tile([128, KT], BF16, tag="hcnbf")
                    nc.vector.tensor_copy(hcn_bf[:], hcn_sb[:])

                    Wgb_s = ws.tile([128, KT * D], BF16, tag="wstream")
                    for k in range(KT):
                        nc.sync.dma_start(out=Wgb_s[:, k * D:(k + 1) * D],
                                          in_=Wgb[k * 128:(k + 1) * 128, :])
                    g0p = ps2.tile([128, KT], F32, tag="g0p")
                    for m in range(KT):
                        for k in range(KT):
                            nc.tensor.matmul(
                                g0p[:, m:m + 1],
                                Wgb_s[:, k * D + m * 128:k * D + (m + 1) * 128],
                                hcn_bf[:, k:k + 1],
                                start=(k == 0), stop=(k == KT - 1))
                    nc.vector.tensor_add(g0_s[:], g0p[:], bgv_s[:])

                    W1b_s = ws.tile([128, KT * D], BF16, tag="wstream")
                    for k in range(KT):
                        nc.sync.dma_start(out=W1b_s[:, k * D:(k + 1) * D],
                                          in_=W1b[k * 128:(k + 1) * 128, :])
                    a0p = ps2.tile([128, KT], F32, tag="a0p")
                    for m in range(KT):
                        for k in range(KT):
                            nc.tensor.matmul(
                                a0p[:, m:m + 1],
                                W1b_s[:, k * D + m * 128:k * D + (m + 1) * 128],
                                hcn_bf[:, k:k + 1],
                                start=(k == 0), stop=(k == KT - 1))
                    nc.vector.tensor_add(a0_s[:], a0p[:], b1v_s[:])

            if variant in ("full", "nocc"):
                # =========================== PASS 2 ===========================
                with (
                    tc.tile_pool(name="p2sb", bufs=2) as sb3,
                    tc.tile_pool(name="p2st", bufs=3) as sb4,
                    tc.tile_pool(name="p2w2", bufs=1) as wres2,
                    tc.tile_pool(name="p2ps", bufs=2, space="PSUM") as ps3,
                ):
                    W2h_s = wres2.tile([128, KT * D], F32R, tag="W2h")
                    for k in range(KT):
                        nc.sync.dma_start(out=W2h_s[:, k * D:(k + 1) * D],
                                          in_=W2h[k * 128:(k + 1) * 128, :])
                    for c in range(NCH):
                        c0 = c * CH
                        hTc = sb3.tile([128, KT * CH], F32R, tag="hTc2")
                        for k in range(KT):
                            nc.sync.dma_start(
                                out=hTc[:, k * CH:(k + 1) * CH],
                                in_=hT[k * 128:(k + 1) * 128, c0:c0 + CH])
                        B = sb3.tile([128, KT * CH], F32R, tag="B")
                        for m in range(KT):
                            A = ps3.tile([128, CH], F32, tag="A")
                            for k in range(KT):
                                nc.tensor.matmul(
                                    A[:], W1t_s[:, k * D + m * 128:k * D + (m + 1) * 128],
                                    hTc[:, k * CH:(k + 1) * CH],
                                    start=(k == 0), stop=(k == KT - 1))
                            nc.scalar.activation(B[:, m * CH:(m + 1) * CH], A[:],
                                                 AF.Silu, bias=a0_s[:, m:m + 1])
                        for m in range(KT):
                            Gt = ps3.tile([128, CH], F32, tag="Gt")
                            for k in range(KT):
                                nc.tensor.matmul(
                                    Gt[:], Wgt_s[:, k * D + m * 128:k * D + (m + 1) * 128],
                                    hTc[:, k * CH:(k + 1) * CH],
                                    start=(k == 0), stop=(k == KT - 1))
                            gs = sb4.tile([128, CH], F32, tag="gs")
                            nc.scalar.activation(gs[:], Gt[:], AF.Sigmoid,
                                                 bias=g0_s[:, m:m + 1])
                            Cp = ps3.tile([128, CH], F32, tag="Cp")
                            for k in range(KT):
                                nc.tensor.matmul(
                                    Cp[:], W2h_s[:, k * D + m * 128:k * D + (m + 1) * 128],
                                    B[:, k * CH:(k + 1) * CH],
                                    start=(k == 0), stop=(k == KT - 1))
                            t6 = sb4.tile([128, CH], F32, tag="t6")
                            nc.vector.scalar_tensor_tensor(
                                t6[:], Cp[:], b2v_s[:, m:m + 1], gs[:],
                                op0=OP.add, op1=OP.mult)
                            ot = sb4.tile([128, CH], F32, tag="ot")
                            nc.vector.tensor_add(
                                ot[:], t6[:],
                                hTc[:, m * CH:(m + 1) * CH].bitcast(F32))
                            nc.sync.dma_start(
                                out=outT[m * 128:(m + 1) * 128, c0:c0 + CH],
                                in_=ot[:])
            wres_cm.__exit__(None, None, None)
    nc.compile()
    return nc


def _get_nc():
    if "nc" not in _CACHE:
        _CACHE["nc"] = _build()
    return _CACHE["nc"]


def kernel(h, center_idx, rbf_ic, seqsep_ic, nbr_idx, local_bias,
           gamma_c, beta_c, gamma_a, beta_a,
           Wq, Wk, Wv, Wo, Wb, W1, b1, W2, b2, Wg, bg):
    global LAST_RESULTS
    f = np.float32
    h = np.asarray(h, f)
    c = int(center_idx)
    rbf_ic = np.asarray(rbf_ic, f)
    seqsep_ic = np.asarray(seqsep_ic, f)
    nbr_idx = np.asarray(nbr_idx)
    local_bias = np.asarray(local_bias, f)
    gamma_c = np.asarray(gamma_c, np.float64)
    beta_c = np.asarray(beta_c, np.float64)
    gamma_a = np.asarray(gamma_a, np.float64)
    beta_a = np.asarray(beta_a, np.float64)
    Wq = np.asarray(Wq, f); Wk = np.asarray(Wk, f); Wv = np.asarray(Wv, f)
    Wo = np.asarray(Wo, f); Wb = np.asarray(Wb, f)
    W1 = np.asarray(W1, f); b1 = np.asarray(b1, f)
    W2 = np.asarray(W2, f); b2 = np.asarray(b2, f)
    Wg = np.asarray(Wg, f); bg = np.asarray(bg, f)

    # ---- host algebra (tiny, no big matmuls) ----
    hc = h[c].astype(np.float64)
    hcl = (hc - hc.mean()) / np.sqrt(hc.var() + EPS) * gamma_c + beta_c
    q = (hcl @ Wq.astype(np.float64)).reshape(H, HD)
    Qm = np.zeros((D, H), np.float64)
    for hh in range(H):
        Qm[hh * HD:(hh + 1) * HD, hh] = q[hh] / np.sqrt(HD)
    Wk1 = Wk.astype(np.float64) @ Qm                    # (D, 16)
    Wkp = (Wk1 * gamma_a[:, None]).astype(f)
    ncg = (-(Wk1 * gamma_a[:, None]).sum(0)).astype(f).reshape(H, 1)
    cbv = (Wk1 * beta_a[:, None]).sum(0).astype(f).reshape(H, 1)

    full_bias = np.zeros((N, local_bias.shape[1]), f)
    full_bias[nbr_idx] = local_bias
    bias_featT = np.ascontiguousarray(
        np.concatenate([rbf_ic, seqsep_ic, full_bias], axis=1).T)  # (128, N)

    hT_full = np.ascontiguousarray(h.T)                 # (D, N)

    gamma_a32 = gamma_a.astype(f)
    beta_a32 = beta_a.astype(f)
    bf = ml_dtypes.bfloat16
    shared = {
        "Wkp": Wkp, "Wb": Wb.astype(bf),
        "W1t": np.ascontiguousarray(W1[:D]),
        "Wgt": np.ascontiguousarray(Wg[:D]),
        "W2h": np.ascontiguousarray(RES * W2),
        "Wv": Wv.astype(bf), "Wo": Wo.astype(bf),
        "W1b": np.ascontiguousarray(W1[D:]).astype(bf),
        "Wgb": np.ascontiguousarray(Wg[D:]).astype(bf),
        "idn": np.eye(128, dtype=f),
        "ones128": np.ones((128, 1), f),
        "ncg": ncg, "cbv": cbv,
        "gb16": np.ascontiguousarray(np.broadcast_to(gamma_a32, (H, D))),
        "bb16": np.ascontiguousarray(np.broadcast_to(beta_a32, (H, D))),
        "hcv": np.ascontiguousarray(h[c].reshape(KT, 128).T),
        "b1v": np.ascontiguousarray(b1.reshape(KT, 128).T),
        "bgv": np.ascontiguousarray(bg.reshape(KT, 128).T),
        "b2v": np.ascontiguousarray((RES * b2).reshape(KT, 128).T),
        "epsv": np.full((1, 1), EPS, f),
    }
    in_maps = []
    for i in range(NCORES):
        r0 = i * NS
        m = dict(shared)
        m["hT"] = np.ascontiguousarray(hT_full[:, r0:r0 + NS])
        m["hN"] = h[r0:r0 + NS].astype(bf)
        m["bT"] = np.ascontiguousarray(bias_featT[:, r0:r0 + NS]).astype(bf)
        in_maps.append(m)

    nc = _get_nc()
    trace = bool(int(os.environ.get("KERNEL_TRACE", "0")))
    res = run_bass_kernel_spmd(nc, in_maps, core_ids=list(range(NCORES)),
                               trace=trace)
    LAST_RESULTS = res

    out = np.empty((N, D), f)
    for i in range(NCORES):
        out[i * NS:(i + 1) * NS] = res.results[i]["outT"].T
    hcn = res.results[0]["outC"].T.reshape(D)           # [m,p] -> flat
    out[c] = hcn
    return out



# revision 14
# speedup vs baseline: 1.8252x; 1.8252x over previous
"""Trainium2 Bass kernel for CenterGeoAttention (N=65536, D=1024, H=16).

Strategy (row-shard N across 8 cores, fp8 streaming + DoubleRow matmuls):

Host algebra reduces the attention almost entirely (same as baseline):
  - q = LN(h[c]) @ Wq folds into Wkp = (Wk @ Qblockdiag) * gamma_a (1024x16).
  - Per-head constant logit terms (beta_a path) drop out of softmax entirely.
  - G = (p*r)^T @ h per core; PRM = rowmean(G) (exact identity), so the
    AllReduce payload is just [G | S] (16 x 1025).
  - h_cat @ W = h @ W_top + rank-1(h_c_new @ W_bot) splits the 2D-wide
    MLP/gate matmuls in half.

Device changes vs the f32r baseline:
  - All big matmuls run fp8(e4m3) with DoubleRow perf mode (2 K-tiles per
    instruction): W1t/Wgt/W2h scaled by 2^k2 on host, descaled for free in
    the ACT scale / STT scalar operands.
  - LN stats are computed once in row-partition layout [128, 64] from a
    resident fp8 copy of hN (DVE row-sum + ACT Square accumulate), so the
    reciprocal is a single [128,64] op instead of 16 x [1,512].
  - The logits pipeline is row-partitioned after a PE transpose; the
    per-row rstd lands as a per-partition STT scalar (no broadcasts).
  - Residual add uses a bf16 h stream; output written f32.
"""

import os
import ml_dtypes
import numpy as np

import concourse.bass as bass
import concourse.bacc as bacc
import concourse.tile as tile
import concourse.mybir as mybir
from concourse.bass_utils import run_bass_kernel_spmd

F32 = mybir.dt.float32
BF16 = mybir.dt.bfloat16
F8 = mybir.dt.float8e4
AF = mybir.ActivationFunctionType
OP = mybir.AluOpType
AX = mybir.AxisListType
DR = mybir.MatmulPerfMode.DoubleRow

NCORES = 8
N, D, H, HD, BIAS = 65536, 1024, 16, 64, 128
NS = N // NCORES            # 8192 rows per core
JB = NS // 128              # 64 row blocks of 128
KT = D // 128               # 8 feature tiles
CH = 1024                   # chunk rows (both passes)
NCH = NS // CH              # 8 chunks
JPC = CH // 128             # 8 row-blocks per chunk
EPS = 1e-5
RES = 0.5

_CACHE = {}
LAST_RESULTS = None


def _build(ncores=NCORES):
    nc = bacc.Bacc("TRN2", target_bir_lowering=False, debug=False,
                   num_devices=ncores)

    def din(name, shape, dt=F8):
        return nc.dram_tensor(name, list(shape), dt, kind="ExternalInput").ap()

    # per-core tensors
    hN8 = din("hN8", (NS, D))             # h shard, row-major fp8
    hT8 = din("hT8", (D, NS))             # h shard^T fp8
    hTb = din("hTb", (D, NS), BF16)       # h shard^T bf16 (residual)
    bT8 = din("bT8", (BIAS, NS))          # bias_feat^T fp8
    # shared weights
    Wkp8 = din("Wkp8", (128, KT, H))      # * 2^k1
    Wb8 = din("Wb8", (BIAS, H))           # unscaled
    W1t8 = din("W1t8", (128, KT, D))      # * 2^k2
    Wgt8 = din("Wgt8", (128, KT, D))      # * 2^k2
    W2h8 = din("W2h8", (128, KT, D))      # RES*W2 * 2^k2
    Wv8 = din("Wv8", (128, KT, D))        # * 2^kv
    Wo8 = din("Wo8", (128, KT, D))        # * 2^ko
    W1b8 = din("W1b8", (128, KT, D))      # * 2^kb
    Wgb8 = din("Wgb8", (128, KT, D))      # * 2^kb
    # small constants
    idn = din("idn", (128, 128), F32)
    ones128b = din("ones128b", (128, 1), BF16)
    cgt = din("cgt", (128, H), F32)       # -cg broadcast rows
    gb16 = din("gb16", (H, D), F32)
    bb16 = din("bb16", (H, D), F32)
    hcv = din("hcv", (128, KT), F32)
    b1v = din("b1v", (128, KT), F32)
    bgv = din("bgv", (128, KT), F32)
    b2v2 = din("b2v2", (128, KT), F32)    # RES*b2*2^k2
    epsv = din("epsv", (128, 1), F32)     # EPS
    sck1 = din("sck1", (128, 1), F32)     # 2^-k1
    sck2 = din("sck2", (128, 1), F32)     # 2^-k2
    scvo = din("scvo", (128, 1), F32)     # RES * 2^-(kv+ko)
    scb = din("scb", (128, 1), F32)       # 2^-kb

    outT = nc.dram_tensor("outT", [D, NS], F32, kind="ExternalOutput").ap()
    outC = nc.dram_tensor("outC", [128, KT], F32, kind="ExternalOutput").ap()

    with tile.TileContext(nc) as tc:
        with (
            tc.tile_pool(name="persist", bufs=1) as pp,
            tc.tile_pool(name="dram", bufs=1, space="DRAM") as dram,
        ):
            # ---- long-lived small tiles ----
            idn_s = pp.tile([128, 128], F32, tag="idn")
            nc.sync.dma_start(out=idn_s[:], in_=idn[:])
            ones_s = pp.tile([128, 1], BF16, tag="ones128b")
            nc.sync.dma_start(out=ones_s[:], in_=ones128b[:])
            cgt_s = pp.tile([128, H], F32, tag="cgt")
            nc.sync.dma_start(out=cgt_s[:], in_=cgt[:])
            hcv_s = pp.tile([128, KT], F32, tag="hcv")
            nc.sync.dma_start(out=hcv_s[:], in_=hcv[:])
            b1v_s = pp.tile([128, KT], F32, tag="b1v")
            nc.sync.dma_start(out=b1v_s[:], in_=b1v[:])
            bgv_s = pp.tile([128, KT], F32, tag="bgv")
            nc.sync.dma_start(out=bgv_s[:], in_=bgv[:])
            b2v_s = pp.tile([128, KT], F32, tag="b2v2")
            nc.sync.dma_start(out=b2v_s[:], in_=b2v2[:])
            epsv_s = pp.tile([128, 1], F32, tag="epsv")
            nc.sync.dma_start(out=epsv_s[:], in_=epsv[:])
            sck1_s = pp.tile([128, 1], F32, tag="sck1")
            nc.sync.dma_start(out=sck1_s[:], in_=sck1[:])
            sck2_s = pp.tile([128, 1], F32, tag="sck2")
            nc.sync.dma_start(out=sck2_s[:], in_=sck2[:])
            scvo_s = pp.tile([128, 1], F32, tag="scvo")
            nc.sync.dma_start(out=scvo_s[:], in_=scvo[:])
            scb_s = pp.tile([128, 1], F32, tag="scb")
            nc.sync.dma_start(out=scb_s[:], in_=scb[:])
            Wkp_s = pp.tile([128, KT, H], F8, tag="Wkp8")
            nc.sync.dma_start(out=Wkp_s[:], in_=Wkp8[:])
            Wb_s = pp.tile([BIAS, H], F8, tag="Wb8")
            nc.sync.dma_start(out=Wb_s[:], in_=Wb8[:])

            # pass-2 + post weights resident (issued early; fp8 = 7MB)
            W1t_s = pp.tile([128, KT, D], F8, tag="W1t8")
            nc.sync.dma_start(out=W1t_s[:], in_=W1t8[:])
            Wgt_s = pp.tile([128, KT, D], F8, tag="Wgt8")
            nc.sync.dma_start(out=Wgt_s[:], in_=Wgt8[:])
            W2h_s = pp.tile([128, KT, D], F8, tag="W2h8")
            nc.sync.dma_start(out=W2h_s[:], in_=W2h8[:])
            Wv_s = pp.tile([128, KT, D], F8, tag="Wv8")
            nc.sync.dma_start(out=Wv_s[:], in_=Wv8[:])
            Wo_s = pp.tile([128, KT, D], F8, tag="Wo8")
            nc.sync.dma_start(out=Wo_s[:], in_=Wo8[:])
            W1b_s = pp.tile([128, KT, D], F8, tag="W1b8")
            nc.sync.dma_start(out=W1b_s[:], in_=W1b8[:])
            Wgb_s = pp.tile([128, KT, D], F8, tag="Wgb8")
            nc.sync.dma_start(out=Wgb_s[:], in_=Wgb8[:])

            # stats + per-row scales (row-partition layout [128, JB])
            r_s = pp.tile([128, JB], F32, tag="r_s")       # rstd
            r2_s = pp.tile([128, JB], F32, tag="r2_s")     # rstd * 2^-k1
            mr_s = pp.tile([128, JB], F32, tag="mr_s")     # mean * rstd
            sCols = pp.tile([1, 128], F32, tag="sCols")    # S partials
            hcn_sb = pp.tile([128, KT], F32, tag="hcn")
            g0_s = pp.tile([128, KT], F32, tag="g0")
            a0_s = pp.tile([128, KT], F32, tag="a0")

            # ================= PASS 1a: resident hN8 + row stats ==========
            hn_cm = tc.tile_pool(name="hn8pool", bufs=1)
            hnp = hn_cm.__enter__()
            hn = hnp.tile([128, JB, D], F8, tag="hn8")
            sum_s = pp.tile([128, JB], F32, tag="sum_s")
            sq_s = pp.tile([128, JB], F32, tag="sq_s")
            with tc.tile_pool(name="sqscr", bufs=2) as scr:
                for j in range(JB):
                    nc.sync.dma_start(out=hn[:, j:j + 1, :],
                                      in_=hN8[j * 128:(j + 1) * 128, :])
                    nc.vector.reduce_sum(sum_s[:, j:j + 1], hn[:, j:j + 1, :],
                                         axis=AX.X)
                    sqt = scr.tile([128, 1, D], BF16, tag="sqt")
                    nc.scalar.activation(sqt[:], hn[:, j:j + 1, :], AF.Square,
                                         accum_out=sq_s[:, j:j + 1])
            # finalize stats
            with tc.tile_pool(name="statfin", bufs=1) as sf, \
                 tc.tile_pool(name="statps", bufs=1, space="PSUM") as sps:
                m_s = sf.tile([128, JB], F32, tag="m_s")
                nc.vector.tensor_scalar_mul(m_s[:], sum_s[:], 1.0 / D)
                msq = sf.tile([128, JB], F32, tag="msq")
                nc.vector.tensor_mul(msq[:], m_s[:], m_s[:])
                var = sf.tile([128, JB], F32, tag="var")
                nc.vector.scalar_tensor_tensor(
                    var[:], sq_s[:], 1.0 / D, msq[:],
                    op0=OP.mult, op1=OP.subtract)
                sd = sf.tile([128, JB], F32, tag="sd")
                nc.scalar.activation(sd[:], var[:], AF.Sqrt,
                                     bias=epsv_s[:, 0:1])
                nc.vector.reciprocal(r_s[:], sd[:])
                nc.vector.tensor_mul(mr_s[:], m_s[:], r_s[:])
                nc.vector.tensor_scalar_mul(r2_s[:], r_s[:], sck1_s[:, 0:1])

            # ================= PASS 1b: logits / softmax / G ==============
            psG_cm = tc.tile_pool(name="p1psG", bufs=1, space="PSUM")
            psG = psG_cm.__enter__()
            G = psG.tile([H, D], F32, tag="G")
            Sp_cm = tc.tile_pool(name="p1psS", bufs=1, space="PSUM")
            Sps = Sp_cm.__enter__()
            Spsum = Sps.tile([1, 128], F32, tag="Spsum")
            with (
                tc.tile_pool(name="p1sb", bufs=2) as sb2,
                tc.tile_pool(name="p1sb1", bufs=2) as sb1,
                tc.tile_pool(name="p1ps", bufs=1, space="PSUM") as ps1,
            ):
                for c in range(NCH):
                    c0 = c * CH
                    jb0 = c * JPC
                    hTc = sb2.tile([128, KT, CH], F8, tag="hTc")
                    for k in range(KT):
                        nc.sync.dma_start(
                            out=hTc[:, k:k + 1, :],
                            in_=hT8[k * 128:(k + 1) * 128, c0:c0 + CH])
                    bTc = sb2.tile([BIAS, CH], F8, tag="bTc")
                    nc.sync.dma_start(out=bTc[:], in_=bT8[:, c0:c0 + CH])

                    # P1 = 2^k1 * (Wkp^T h) : DoubleRow over k-pairs
                    P1 = ps1.tile([H, CH], F32, tag="P1")
                    for half in range(2):
                        for kp in range(KT // 2):
                            nc.tensor.matmul(
                                P1[:, half * 512:(half + 1) * 512],
                                Wkp_s[:, 2 * kp:2 * kp + 2, :],
                                hTc[:, 2 * kp:2 * kp + 2,
                                    half * 512:(half + 1) * 512],
                                start=(kp == 0), stop=(kp == KT // 2 - 1),
                                perf_mode=DR)
                    # L2rt[j] = (bias_feat @ Wb) row-partitioned
                    L2rt = ps1.tile([128, JPC, H], F32, tag="L2rt")
                    for j in range(JPC):
                        nc.tensor.matmul(
                            L2rt[:, j:j + 1, :],
                            bTc[:, j * 128:(j + 1) * 128],
                            Wb_s[:],
                            start=True, stop=True)
                    # transpose P1 to row-partition (PE needs SBUF input)
                    P1c = sb1.tile([H, CH], F32, tag="P1c")
                    nc.vector.tensor_copy(P1c[:], P1[:])
                    tp = ps1.tile([128, JPC, H], F32, tag="tp")
                    for j in range(JPC):
                        nc.tensor.transpose(
                            tp[:, j:j + 1, :],
                            P1c[:, j * 128:(j + 1) * 128],
                            idn_s[0:H, 0:H])
                    # t5 = r*(L - cg*m) + L2 ; p = exp(t5); pr = p*r
                    L2c = sb1.tile([128, JPC, H], F32, tag="L2c")
                    nc.vector.tensor_copy(L2c[:], L2rt[:])
                    t5 = sb1.tile([128, JPC, H], F32, tag="t5")
                    p_sb = sb1.tile([128, JPC, H], BF16, tag="p_sb")
                    pr8 = sb1.tile([128, JPC, H], F8, tag="pr8")
                    for j in range(JPC):
                        jb = jb0 + j
                        nc.vector.scalar_tensor_tensor(
                            t5[:, j:j + 1, :], tp[:, j:j + 1, :],
                            r2_s[:, jb:jb + 1], L2c[:, j:j + 1, :],
                            op0=OP.mult, op1=OP.add)
                        nc.vector.scalar_tensor_tensor(
                            t5[:, j:j + 1, :], cgt_s[:],
                            mr_s[:, jb:jb + 1], t5[:, j:j + 1, :],
                            op0=OP.mult, op1=OP.add)
                        nc.scalar.activation(p_sb[:, j:j + 1, :],
                                             t5[:, j:j + 1, :], AF.Exp)
                        nc.vector.tensor_scalar_mul(
                            pr8[:, j:j + 1, :], p_sb[:, j:j + 1, :],
                            r_s[:, jb:jb + 1])
                    # S partials: ones^T @ p  (accumulated over chunks)
                    nc.tensor.matmul(Spsum[:], ones_s[:],
                                     p_sb[:, :, :],
                                     start=(c == 0), stop=(c == NCH - 1))
                    # G += pr^T @ h (DoubleRow over row-block pairs)
                    for half in range(2):
                        for jp in range(JPC // 2):
                            nc.tensor.matmul(
                                G[:, half * 512:(half + 1) * 512],
                                pr8[:, 2 * jp:2 * jp + 2, :],
                                hn[:, jb0 + 2 * jp:jb0 + 2 * jp + 2,
                                   half * 512:(half + 1) * 512],
                                start=(c == 0 and jp == 0),
                                stop=(c == NCH - 1 and jp == JPC // 2 - 1),
                                perf_mode=DR)

                Gacc = pp.tile([H, D], F32, tag="Gacc")
                nc.vector.tensor_copy(Gacc[:], G[:])
                # S16 = sum over j-positions of Spsum [1, (j,h)]
                Scp = pp.tile([1, JPC * H], F32, tag="Scp")
                nc.vector.tensor_copy(Scp[:], Spsum[:])
                Ssum = pp.tile([1, H], F32, tag="Ssum")
                nc.vector.tensor_add(Ssum[:], Scp[0:1, 0:H],
                                     Scp[0:1, H:2 * H])
                for j in range(2, JPC):
                    nc.vector.tensor_add(Ssum[:], Ssum[:],
                                         Scp[0:1, j * H:(j + 1) * H])

            Sp_cm.__exit__(None, None, None)
            psG_cm.__exit__(None, None, None)
            hn_cm.__exit__(None, None, None)

            # ---- AllReduce [G | S] ----
            with tc.tile_pool(name="postps0", bufs=1, space="PSUM") as ps0:
                S16p = ps0.tile([H, 1], F32, tag="S16p")
                nc.tensor.transpose(S16p[:], Ssum[:], idn_s[0:1, 0:1])
                S16 = pp.tile([H, 1], F32, tag="S16")
                nc.vector.tensor_copy(S16[:], S16p[:])

            arin = dram.tile([H, D + 1], F32, tag="arin")
            arout = dram.tile([H, D + 1], F32, tag="arout")
            nc.sync.dma_start(out=arin[:, 0:D], in_=Gacc[:])
            nc.sync.dma_start(out=arin[:, D:D + 1], in_=S16[:])
            nc.gpsimd.collective_compute(
                "AllReduce", OP.add,
                replica_groups=[list(range(ncores))],
                ins=[arin.opt()], outs=[arout.opt()])

            # ---- post: h_c_new, a0, g0 ----
            with (
                tc.tile_pool(name="postsb", bufs=1) as psb,
                tc.tile_pool(name="postps", bufs=1, space="PSUM") as ps2,
            ):
                gb16_s = psb.tile([H, D], F32, tag="gb16")
                nc.sync.dma_start(out=gb16_s[:], in_=gb16[:])
                bb16_s = psb.tile([H, D], F32, tag="bb16")
                nc.sync.dma_start(out=bb16_s[:], in_=bb16[:])
                Gar = psb.tile([H, D], F32, tag="Gar")
                nc.sync.dma_start(out=Gar[:], in_=arout[:, 0:D])
                Sar = psb.tile([H, 1], F32, tag="Sar")
                nc.sync.dma_start(out=Sar[:], in_=arout[:, D:D + 1])
                # PRM = rowsum(G)/D  (exact identity)
                PRMt = psb.tile([H, 1], F32, tag="PRMt")
                nc.vector.reduce_sum(PRMt[:], Gar[:], axis=AX.X)
                nc.vector.tensor_scalar_mul(PRMt[:], PRMt[:], 1.0 / D)
                Gn = psb.tile([H, D], F32, tag="Gn")
                nc.vector.tensor_scalar_sub(Gn[:], Gar[:], PRMt[:, 0:1])
                nc.vector.tensor_mul(Gn[:], Gn[:], gb16_s[:])
                nc.vector.scalar_tensor_tensor(
                    Gn[:], bb16_s[:], Sar[:, 0:1], Gn[:],
                    op0=OP.mult, op1=OP.add)
                sr = psb.tile([H, 1], F32, tag="sr")
                nc.vector.reciprocal(sr[:], Sar[:])
                nc.vector.tensor_scalar_mul(Gn[:], Gn[:], sr[:, 0:1])

                tpg = ps2.tile([128, KT, H], F32, tag="tpg")
                for m in range(KT):
                    nc.tensor.transpose(
                        tpg[:, m:m + 1, :],
                        Gn[:, m * 128:(m + 1) * 128],
                        idn_s[0:H, 0:H])
                GnT = psb.tile([128, KT, H], F8, tag="GnT")
                nc.vector.tensor_copy(GnT[:], tpg[:])

                # out_center block-diag of (Gn @ Wv), scaled 2^kv
                OCp = ps2.tile([128, KT, H], F32, tag="OCp")
                for m in range(KT):
                    for k in range(KT):
                        nc.tensor.matmul(
                            OCp[:, m:m + 1, :],
                            Wv_s[:, k:k + 1, m * 128:(m + 1) * 128],
                            GnT[:, k:k + 1, :],
                            start=(k == 0), stop=(k == KT - 1))
                ocv = pp.tile([128, KT], F8, tag="ocv")
                for m in range(KT):
                    nc.vector.tensor_copy(
                        ocv[0:64, m:m + 1],
                        OCp[0:64, m:m + 1, 2 * m:2 * m + 1])
                    nc.vector.tensor_copy(
                        ocv[64:128, m:m + 1],
                        OCp[64:128, m:m + 1, 2 * m + 1:2 * m + 2])

                hcp = ps2.tile([128, KT], F32, tag="hcp")
                for m in range(KT):
                    for k in range(KT):
                        nc.tensor.matmul(
                            hcp[:, m:m + 1],
                            Wo_s[:, k:k + 1, m * 128:(m + 1) * 128],
                            ocv[:, k:k + 1],
                            start=(k == 0), stop=(k == KT - 1))
                nc.vector.scalar_tensor_tensor(
                    hcn_sb[:], hcp[:], scvo_s[:, 0:1], hcv_s[:],
                    op0=OP.mult, op1=OP.add)
                nc.sync.dma_start(out=outC[:], in_=hcn_sb[:])
                hcn8 = pp.tile([128, KT], F8, tag="hcn8")
                nc.vector.tensor_copy(hcn8[:], hcn_sb[:])

                g0p = ps2.tile([128, KT], F32, tag="g0p")
                for m in range(KT):
                    for k in range(KT):
                        nc.tensor.matmul(
                            g0p[:, m:m + 1],
                            Wgb_s[:, k:k + 1, m * 128:(m + 1) * 128],
                            hcn8[:, k:k + 1],
                            start=(k == 0), stop=(k == KT - 1))
                nc.vector.scalar_tensor_tensor(
                    g0_s[:], g0p[:], scb_s[:, 0:1], bgv_s[:],
                    op0=OP.mult, op1=OP.add)
                a0p = ps2.tile([128, KT], F32, tag="a0p")
                for m in range(KT):
                    for k in range(KT):
                        nc.tensor.matmul(
                            a0p[:, m:m + 1],
                            W1b_s[:, k:k + 1, m * 128:(m + 1) * 128],
                            hcn8[:, k:k + 1],
                            start=(k == 0), stop=(k == KT - 1))
                nc.vector.scalar_tensor_tensor(
                    a0_s[:], a0p[:], scb_s[:, 0:1], b1v_s[:],
                    op0=OP.mult, op1=OP.add)

            # ========================= PASS 2 =============================
            with (
                tc.tile_pool(name="p2sb", bufs=2) as sb3,
                tc.tile_pool(name="p2st", bufs=3) as sb4,
                tc.tile_pool(name="p2ps", bufs=2, space="PSUM") as ps3,
            ):
                for c in range(NCH):
                    c0 = c * CH
                    hTc2 = sb3.tile([128, KT, CH], F8, tag="hTc2")
                    for k in range(KT):
                        nc.sync.dma_start(
                            out=hTc2[:, k:k + 1, :],
                            in_=hT8[k * 128:(k + 1) * 128, c0:c0 + CH])
                    hTbc = sb3.tile([128, KT, CH], BF16, tag="hTbc")
                    for k in range(KT):
                        nc.sync.dma_start(
                            out=hTbc[:, k:k + 1, :],
                            in_=hTb[k * 128:(k + 1) * 128, c0:c0 + CH])
                    B8 = sb3.tile([128, KT, CH], F8, tag="B8")
                    for m in range(KT):
                        for half in range(2):
                            A = ps3.tile([128, 512], F32, tag="A")
                            for kp in range(KT // 2):
                                nc.tensor.matmul(
                                    A[:],
                                    W1t_s[:, 2 * kp:2 * kp + 2,
                                          m * 128:(m + 1) * 128],
                                    hTc2[:, 2 * kp:2 * kp + 2,
                                         half * 512:(half + 1) * 512],
                                    start=(kp == 0), stop=(kp == KT // 2 - 1),
                                    perf_mode=DR)
                            nc.scalar.activation(
                                B8[:, m:m + 1, half * 512:(half + 1) * 512],
                                A[:], AF.Silu,
                                bias=a0_s[:, m:m + 1], scale=sck2_s[:, 0:1])
                    for m in range(KT):
                        for half in range(2):
                            Gt = ps3.tile([128, 512], F32, tag="Gt")
                            for kp in range(KT // 2):
                                nc.tensor.matmul(
                                    Gt[:],
                                    Wgt_s[:, 2 * kp:2 * kp + 2,
                                          m * 128:(m + 1) * 128],
                                    hTc2[:, 2 * kp:2 * kp + 2,
                                         half * 512:(half + 1) * 512],
                                    start=(kp == 0), stop=(kp == KT // 2 - 1),
                                    perf_mode=DR)
                            gs = sb4.tile([128, 512], BF16, tag="gs")
                            nc.scalar.activation(
                                gs[:], Gt[:], AF.Sigmoid,
                                bias=g0_s[:, m:m + 1], scale=sck2_s[:, 0:1])
                            Cp = ps3.tile([128, 512], F32, tag="Cp")
                            for kp in range(KT // 2):
                                nc.tensor.matmul(
                                    Cp[:],
                                    W2h_s[:, 2 * kp:2 * kp + 2,
                                          m * 128:(m + 1) * 128],
                                    B8[:, 2 * kp:2 * kp + 2,
                                       half * 512:(half + 1) * 512],
                                    start=(kp == 0), stop=(kp == KT // 2 - 1),
                                    perf_mode=DR)
                            u = sb4.tile([128, 512], F32, tag="u")
                            nc.vector.scalar_tensor_tensor(
                                u[:], Cp[:], b2v_s[:, m:m + 1], gs[:],
                                op0=OP.add, op1=OP.mult)
                            ot = sb4.tile([128, 512], F32, tag="ot")
                            nc.vector.scalar_tensor_tensor(
                                ot[:], u[:], sck2_s[:, 0:1],
                                hTbc[:, m:m + 1, half * 512:(half + 1) * 512],
                                op0=OP.mult, op1=OP.add)
                            nc.sync.dma_start(
                                out=outT[m * 128:(m + 1) * 128,
                                         c0 + half * 512:c0 + (half + 1) * 512],
                                in_=ot[:])
    nc.compile()
    return nc


def _get_nc():
    if "nc" not in _CACHE:
        _CACHE["nc"] = _build()
    return _CACHE["nc"]


def _exp_scale(maxval, target=96.0):
    if maxval <= 0:
        return 0
    return int(np.floor(np.log2(target / maxval)))


def kernel(h, center_idx, rbf_ic, seqsep_ic, nbr_idx, local_bias,
           gamma_c, beta_c, gamma_a, beta_a,
           Wq, Wk, Wv, Wo, Wb, W1, b1, W2, b2, Wg, bg):
    global LAST_RESULTS
    f = np.float32
    f8 = ml_dtypes.float8_e4m3
    bf = ml_dtypes.bfloat16
    h = np.asarray(h, f)
    c = int(center_idx)
    rbf_ic = np.asarray(rbf_ic, f)
    seqsep_ic = np.asarray(seqsep_ic, f)
    nbr_idx = np.asarray(nbr_idx)
    local_bias = np.asarray(local_bias, f)
    gamma_c = np.asarray(gamma_c, np.float64)
    beta_c = np.asarray(beta_c, np.float64)
    gamma_a = np.asarray(gamma_a, np.float64)
    beta_a = np.asarray(beta_a, np.float64)
    Wq = np.asarray(Wq, f); Wk = np.asarray(Wk, f); Wv = np.asarray(Wv, f)
    Wo = np.asarray(Wo, f); Wb = np.asarray(Wb, f)
    W1 = np.asarray(W1, f); b1 = np.asarray(b1, f)
    W2 = np.asarray(W2, f); b2 = np.asarray(b2, f)
    Wg = np.asarray(Wg, f); bg = np.asarray(bg, f)

    # ---- host algebra (tiny, no big matmuls) ----
    hc = h[c].astype(np.float64)
    hcl = (hc - hc.mean()) / np.sqrt(hc.var() + EPS) * gamma_c + beta_c
    q = (hcl @ Wq.astype(np.float64)).reshape(H, HD)
    Qm = np.zeros((D, H), np.float64)
    for hh in range(H):
        Qm[hh * HD:(hh + 1) * HD, hh] = q[hh] / np.sqrt(HD)
    Wk1 = Wk.astype(np.float64) @ Qm                    # (D, 16)
    Wkp = (Wk1 * gamma_a[:, None]).astype(f)            # (D, 16)
    cg = Wkp.sum(0)                                     # (16,)

    full_bias = np.zeros((N, local_bias.shape[1]), f)
    full_bias[nbr_idx] = local_bias
    bias_featT = np.ascontiguousarray(
        np.concatenate([rbf_ic, seqsep_ic, full_bias], axis=1).T)  # (128, N)

    # scale exponents
    k1 = _exp_scale(np.abs(Wkp).max())
    k2 = _exp_scale(max(np.abs(W1[:D]).max(), np.abs(Wg[:D]).max(),
                        np.abs(RES * W2).max()))
    kv = _exp_scale(np.abs(Wv).max())
    ko = _exp_scale(np.abs(Wo).max())
    kb = _exp_scale(max(np.abs(W1[D:]).max(), np.abs(Wg[D:]).max()))

    def pack3(W, s):
        # (D, Dout) -> [128, KT, Dout] scaled fp8
        return np.ascontiguousarray(
            (W * s).reshape(KT, 128, -1).transpose(1, 0, 2)).astype(f8)

    gamma_a32 = gamma_a.astype(f)
    beta_a32 = beta_a.astype(f)
    shared = {
        "Wkp8": pack3(Wkp, 2.0 ** k1),
        "Wb8": Wb.astype(f8),
        "W1t8": pack3(W1[:D], 2.0 ** k2),
        "Wgt8": pack3(Wg[:D], 2.0 ** k2),
        "W2h8": pack3(RES * W2, 2.0 ** k2),
        "Wv8": pack3(Wv, 2.0 ** kv),
        "Wo8": pack3(Wo, 2.0 ** ko),
        "W1b8": pack3(W1[D:], 2.0 ** kb),
        "Wgb8": pack3(Wg[D:], 2.0 ** kb),
        "idn": np.eye(128, dtype=f),
        "ones128b": np.ones((128, 1), bf),
        "cgt": np.ascontiguousarray(np.broadcast_to(-cg, (128, H))),
        "gb16": np.ascontiguousarray(np.broadcast_to(gamma_a32, (H, D))),
        "bb16": np.ascontiguousarray(np.broadcast_to(beta_a32, (H, D))),
        "hcv": np.ascontiguousarray(h[c].reshape(KT, 128).T),
        "b1v": np.ascontiguousarray(b1.reshape(KT, 128).T),
        "bgv": np.ascontiguousarray(bg.reshape(KT, 128).T),
        "b2v2": np.ascontiguousarray(
            (RES * b2 * 2.0 ** k2).reshape(KT, 128).T),
        "epsv": np.full((128, 1), EPS, f),
        "sck1": np.full((128, 1), 2.0 ** -k1, f),
        "sck2": np.full((128, 1), 2.0 ** -k2, f),
        "scvo": np.full((128, 1), RES * 2.0 ** -(kv + ko), f),
        "scb": np.full((128, 1), 2.0 ** -kb, f),
    }
    in_maps = []
    for i in range(NCORES):
        r0 = i * NS
        sh = h[r0:r0 + NS]
        shT = np.ascontiguousarray(sh.T)
        m = dict(shared)
        m["hN8"] = sh.astype(f8)
        m["hT8"] = shT.astype(f8)
        m["hTb"] = shT.astype(bf)
        m["bT8"] = np.ascontiguousarray(
            bias_featT[:, r0:r0 + NS]).astype(f8)
        in_maps.append(m)

    nc = _get_nc()
    trace = bool(int(os.environ.get("KERNEL_TRACE", "0")))
    res = run_bass_kernel_spmd(nc, in_maps, core_ids=list(range(NCORES)),
                               trace=trace)
    LAST_RESULTS = res

    out = np.empty((N, D), f)
    for i in range(NCORES):
        out[i * NS:(i + 1) * NS] = res.results[i]["outT"].T
    hcn = res.results[0]["outC"].T.reshape(D)           # [m,p] -> flat
    out[c] = hcn
    return out


# revision 36
# speedup vs baseline: 1.9059x; 1.0442x over previous
"""Trainium2 Bass kernel for CenterGeoAttention (N=65536, D=1024, H=16).

Strategy (row-shard N across 8 cores, fp8 streaming + DoubleRow matmuls):

Host algebra reduces the attention almost entirely (same as baseline):
  - q = LN(h[c]) @ Wq folds into Wkp = (Wk @ Qblockdiag) * gamma_a (1024x16).
  - Per-head constant logit terms (beta_a path) drop out of softmax entirely.
  - G = (p*r)^T @ h per core; PRM = rowmean(G) (exact identity), so the
    AllReduce payload is just [G | S] (16 x 1025).
  - h_cat @ W = h @ W_top + rank-1(h_c_new @ W_bot) splits the 2D-wide
    MLP/gate matmuls in half.

Device changes vs the f32r baseline:
  - All big matmuls run fp8(e4m3) with DoubleRow perf mode (2 K-tiles per
    instruction): W1t/Wgt/W2h scaled by 2^k2 on host, descaled for free in
    the ACT scale / STT scalar operands.
  - LN stats are computed once in row-partition layout [128, 64] from a
    resident fp8 copy of hN (DVE row-sum + ACT Square accumulate), so the
    reciprocal is a single [128,64] op instead of 16 x [1,512].
  - The logits pipeline is row-partitioned after a PE transpose; the
    per-row rstd lands as a per-partition STT scalar (no broadcasts).
  - Residual add uses a bf16 h stream; output written f32.
"""

import os
import ml_dtypes
import numpy as np

import concourse.bass as bass
import concourse.bacc as bacc
import concourse.tile as tile
import concourse.mybir as mybir
from concourse.bass_utils import run_bass_kernel_spmd

F32 = mybir.dt.float32
BF16 = mybir.dt.bfloat16
F8 = mybir.dt.float8e4
AF = mybir.ActivationFunctionType
OP = mybir.AluOpType
AX = mybir.AxisListType
DR = mybir.MatmulPerfMode.DoubleRow

NCORES = 8
N, D, H, HD, BIAS = 65536, 1024, 16, 64, 128
NS = N // NCORES            # 8192 rows per core
JB = NS // 128              # 64 row blocks of 128
KT = D // 128               # 8 feature tiles
CH = 1024                   # chunk rows (both passes)
NCH = NS // CH              # 8 chunks
JPC = CH // 128             # 8 row-blocks per chunk
EPS = 1e-5
RES = 0.5

_CACHE = {}
LAST_RESULTS = None


def _build(ncores=NCORES):
    nc = bacc.Bacc("TRN2", target_bir_lowering=False, debug=False,
                   num_devices=ncores)

    def din(name, shape, dt=F8):
        return nc.dram_tensor(name, list(shape), dt, kind="ExternalInput").ap()

    # per-core tensors
    hN8 = din("hN8", (NS, D))             # h shard, row-major fp8
    hT8 = din("hT8", (D, NS))             # h shard^T fp8
    hTb = din("hTb", (D, NS), BF16)       # h shard^T bf16 (residual)
    bT8 = din("bT8", (BIAS, NS))          # bias_feat^T fp8
    # shared weights
    Wkp8 = din("Wkp8", (128, KT, H))      # * 2^k1
    Wb8 = din("Wb8", (BIAS, H))           # unscaled
    W1t8 = din("W1t8", (128, KT, D))      # * 2^k2
    Wgt8 = din("Wgt8", (128, KT, D))      # * 2^k2
    W2h8 = din("W2h8", (128, KT, D))      # RES*W2 * 2^k2
    Wv8 = din("Wv8", (128, KT, D))        # * 2^kv
    Wo8 = din("Wo8", (128, KT, D))        # * 2^ko
    W1b8 = din("W1b8", (128, KT, D))      # * 2^kb
    Wgb8 = din("Wgb8", (128, KT, D))      # * 2^kb
    # small constants
    idn = din("idn", (128, 128), F32)
    ones128b = din("ones128b", (128, 1), BF16)
    cgt = din("cgt", (128, H), F32)       # -cg broadcast rows
    gb16 = din("gb16", (H, D), F32)
    bb16 = din("bb16", (H, D), F32)
    hcv = din("hcv", (128, KT), F32)
    b1v = din("b1v", (128, KT), F32)
    bgv = din("bgv", (128, KT), F32)
    b2v2 = din("b2v2", (128, KT), F32)    # RES*b2*2^k2
    epsv = din("epsv", (128, 1), F32)     # EPS
    sck1 = din("sck1", (128, 1), F32)     # 2^-k1
    sck2 = din("sck2", (128, 1), F32)     # 2^-k2
    scvo = din("scvo", (128, 1), F32)     # RES * 2^-(kv+ko)
    scb = din("scb", (128, 1), F32)       # 2^-kb

    outT = nc.dram_tensor("outT", [D, NS], BF16, kind="ExternalOutput").ap()
    outC = nc.dram_tensor("outC", [128, KT], F32, kind="ExternalOutput").ap()

    with tile.TileContext(nc) as tc:
        with (
            tc.tile_pool(name="persist", bufs=1) as pp,
            tc.tile_pool(name="dram", bufs=1, space="DRAM") as dram,
        ):
            # ---- long-lived small tiles ----
            idn_s = pp.tile([128, 128], F32, tag="idn")
            nc.sync.dma_start(out=idn_s[:], in_=idn[:])
            ones_s = pp.tile([128, 1], BF16, tag="ones128b")
            nc.sync.dma_start(out=ones_s[:], in_=ones128b[:])
            cgt_s = pp.tile([128, H], F32, tag="cgt")
            nc.sync.dma_start(out=cgt_s[:], in_=cgt[:])
            hcv_s = pp.tile([128, KT], F32, tag="hcv")
            nc.sync.dma_start(out=hcv_s[:], in_=hcv[:])
            b1v_s = pp.tile([128, KT], F32, tag="b1v")
            nc.sync.dma_start(out=b1v_s[:], in_=b1v[:])
            bgv_s = pp.tile([128, KT], F32, tag="bgv")
            nc.sync.dma_start(out=bgv_s[:], in_=bgv[:])
            b2v_s = pp.tile([128, KT], F32, tag="b2v2")
            nc.sync.dma_start(out=b2v_s[:], in_=b2v2[:])
            epsv_s = pp.tile([128, 1], F32, tag="epsv")
            nc.sync.dma_start(out=epsv_s[:], in_=epsv[:])
            sck1_s = pp.tile([128, 1], F32, tag="sck1")
            nc.sync.dma_start(out=sck1_s[:], in_=sck1[:])
            sck2_s = pp.tile([128, 1], F32, tag="sck2")
            nc.sync.dma_start(out=sck2_s[:], in_=sck2[:])
            scvo_s = pp.tile([128, 1], F32, tag="scvo")
            nc.sync.dma_start(out=scvo_s[:], in_=scvo[:])
            scb_s = pp.tile([128, 1], F32, tag="scb")
            nc.sync.dma_start(out=scb_s[:], in_=scb[:])
            Wkp_s = pp.tile([128, KT, H], F8, tag="Wkp8")
            nc.sync.dma_start(out=Wkp_s[:], in_=Wkp8[:])
            Wb_s = pp.tile([BIAS, H], F8, tag="Wb8")
            nc.sync.dma_start(out=Wb_s[:], in_=Wb8[:])

            # pass-2 + post weights resident (issued early; fp8 = 7MB)
            W1t_s = pp.tile([128, KT, D], F8, tag="W1t8")
            nc.sync.dma_start(out=W1t_s[:], in_=W1t8[:])
            Wgt_s = pp.tile([128, KT, D], F8, tag="Wgt8")
            nc.sync.dma_start(out=Wgt_s[:], in_=Wgt8[:])
            W2h_s = pp.tile([128, KT, D], F8, tag="W2h8")
            nc.sync.dma_start(out=W2h_s[:], in_=W2h8[:])
            Wv_s = pp.tile([128, KT, D], F8, tag="Wv8")
            nc.sync.dma_start(out=Wv_s[:], in_=Wv8[:])
            Wo_s = pp.tile([128, KT, D], F8, tag="Wo8")
            nc.sync.dma_start(out=Wo_s[:], in_=Wo8[:])
            W1b_s = pp.tile([128, KT, D], F8, tag="W1b8")
            nc.sync.dma_start(out=W1b_s[:], in_=W1b8[:])
            Wgb_s = pp.tile([128, KT, D], F8, tag="Wgb8")
            nc.sync.dma_start(out=Wgb_s[:], in_=Wgb8[:])

            # stats + per-row scales (row-partition layout [128, JB])
            r_s = pp.tile([128, JB], F32, tag="r_s")       # rstd
            r2_s = pp.tile([128, JB], F32, tag="r2_s")     # rstd * 2^-k1
            mr_s = pp.tile([128, JB], F32, tag="mr_s")     # mean * rstd
            sCols = pp.tile([1, 128], F32, tag="sCols")    # S partials
            hcn_sb = pp.tile([128, KT], F32, tag="hcn")
            g0_s = pp.tile([128, KT], F32, tag="g0")
            a0_s = pp.tile([128, KT], F32, tag="a0")

            # ================= PASS 1a: resident hN (fp8) + row stats ====
            # sumsq on ACT via Square-accum (one table load); sums split
            # between DVE and GpSimd tensor_reduce so no single engine
            # serializes the 16MB-equivalent sweep.
            hn_cm = tc.tile_pool(name="hnpool", bufs=1)
            hnp = hn_cm.__enter__()
            hn = hnp.tile([128, JB, D], F8, tag="hn8")
            sum_s = pp.tile([128, JB], F32, tag="sum_s")
            sq_s = pp.tile([128, JB], F32, tag="sq_s")
            with tc.tile_pool(name="sqscr", bufs=3) as scr:
                for j in range(JB):
                    nc.sync.dma_start(out=hn[:, j:j + 1, :],
                                      in_=hN8[j * 128:(j + 1) * 128, :])
                    sqt = scr.tile([128, 1, D], BF16, tag="sqt")
                    nc.scalar.activation(sqt[:], hn[:, j:j + 1, :], AF.Square,
                                         accum_out=sq_s[:, j:j + 1])
                    nc.vector.reduce_sum(sum_s[:, j:j + 1], hn[:, j:j + 1, :],
                                         axis=AX.X)
            # finalize stats
            with tc.tile_pool(name="statfin", bufs=1) as sf, \
                 tc.tile_pool(name="statps", bufs=1, space="PSUM") as sps:
                m_s = sf.tile([128, JB], F32, tag="m_s")
                nc.vector.tensor_scalar_mul(m_s[:], sum_s[:], 1.0 / D)
                msq = sf.tile([128, JB], F32, tag="msq")
                nc.vector.tensor_mul(msq[:], m_s[:], m_s[:])
                var = sf.tile([128, JB], F32, tag="var")
                nc.vector.scalar_tensor_tensor(
                    var[:], sq_s[:], 1.0 / D, msq[:],
                    op0=OP.mult, op1=OP.subtract)
                sd = sf.tile([128, JB], F32, tag="sd")
                nc.scalar.activation(sd[:], var[:], AF.Sqrt,
                                     bias=epsv_s[:, 0:1])
                nc.vector.reciprocal(r_s[:], sd[:])
                nc.vector.tensor_mul(mr_s[:], m_s[:], r_s[:])
                nc.vector.tensor_scalar_mul(r2_s[:], r_s[:], sck1_s[:, 0:1])

            # ================= PASS 1b: logits / softmax / G ==============
            psG_cm = tc.tile_pool(name="p1psG", bufs=1, space="PSUM")
            psG = psG_cm.__enter__()
            G = psG.tile([H, D], F32, tag="G")
            Sp_cm = tc.tile_pool(name="p1psS", bufs=1, space="PSUM")
            Sps = Sp_cm.__enter__()
            Spsum = Sps.tile([1, 128], F32, tag="Spsum")
            with (
                tc.tile_pool(name="p1sb", bufs=2) as sb2,
                tc.tile_pool(name="p1sb1", bufs=2) as sb1,
                tc.tile_pool(name="p1ps", bufs=1, space="PSUM") as ps1,
            ):
                for c in range(NCH):
                    c0 = c * CH
                    jb0 = c * JPC
                    hTc = sb2.tile([128, KT, CH], F8, tag="hTc")
                    for k in range(KT):
                        nc.sync.dma_start(
                            out=hTc[:, k:k + 1, :],
                            in_=hT8[k * 128:(k + 1) * 128, c0:c0 + CH])
                    bTc = sb2.tile([BIAS, CH], F8, tag="bTc")
                    nc.sync.dma_start(out=bTc[:], in_=bT8[:, c0:c0 + CH])

                    # P1 = 2^k1 * (Wkp^T h) : DoubleRow over k-pairs
                    P1 = ps1.tile([H, CH], F32, tag="P1")
                    for half in range(2):
                        for kp in range(KT // 2):
                            nc.tensor.matmul(
                                P1[:, half * 512:(half + 1) * 512],
                                Wkp_s[:, 2 * kp:2 * kp + 2, :],
                                hTc[:, 2 * kp:2 * kp + 2,
                                    half * 512:(half + 1) * 512],
                                start=(kp == 0), stop=(kp == KT // 2 - 1),
                                perf_mode=DR)
                    # L2rt[j] = (bias_feat @ Wb) row-partitioned
                    L2rt = ps1.tile([128, JPC, H], F32, tag="L2rt")
                    for j in range(JPC):
                        nc.tensor.matmul(
                            L2rt[:, j:j + 1, :],
                            bTc[:, j * 128:(j + 1) * 128],
                            Wb_s[:],
                            start=True, stop=True)
                    # transpose P1 to row-partition (PE needs SBUF input)
                    P1c = sb1.tile([H, CH], F32, tag="P1c")
                    nc.vector.tensor_copy(P1c[:], P1[:])
                    tp = ps1.tile([128, JPC, H], F32, tag="tp")
                    for j in range(JPC):
                        nc.tensor.transpose(
                            tp[:, j:j + 1, :],
                            P1c[:, j * 128:(j + 1) * 128],
                            idn_s[0:H, 0:H])
                    # t5 = r*(L - cg*m) + L2 ; p = exp(t5); pr = p*r
                    L2c = sb1.tile([128, JPC, H], F32, tag="L2c")
                    nc.vector.tensor_copy(L2c[:], L2rt[:])
                    t5 = sb1.tile([128, JPC, H], F32, tag="t5")
                    p_sb = sb1.tile([128, JPC, H], BF16, tag="p_sb")
                    prb = sb1.tile([128, JPC, H], F8, tag="prb")
                    for j in range(JPC):
                        jb = jb0 + j
                        nc.vector.scalar_tensor_tensor(
                            t5[:, j:j + 1, :], tp[:, j:j + 1, :],
                            r2_s[:, jb:jb + 1], L2c[:, j:j + 1, :],
                            op0=OP.mult, op1=OP.add)
                        nc.vector.scalar_tensor_tensor(
                            t5[:, j:j + 1, :], cgt_s[:],
                            mr_s[:, jb:jb + 1], t5[:, j:j + 1, :],
                            op0=OP.mult, op1=OP.add)
                        nc.scalar.activation(p_sb[:, j:j + 1, :],
                                             t5[:, j:j + 1, :], AF.Exp)
                        nc.vector.tensor_scalar_mul(
                            prb[:, j:j + 1, :], p_sb[:, j:j + 1, :],
                            r_s[:, jb:jb + 1])
                    # S partials: ones^T @ p  (accumulated over chunks)
                    nc.tensor.matmul(Spsum[:], ones_s[:],
                                     p_sb[:, :, :],
                                     start=(c == 0), stop=(c == NCH - 1))
                    # G += pr^T @ h (DoubleRow over row-block pairs)
                    for half in range(2):
                        for jp in range(JPC // 2):
                            nc.tensor.matmul(
                                G[:, half * 512:(half + 1) * 512],
                                prb[:, 2 * jp:2 * jp + 2, :],
                                hn[:, jb0 + 2 * jp:jb0 + 2 * jp + 2,
                                   half * 512:(half + 1) * 512],
                                start=(c == 0 and jp == 0),
                                stop=(c == NCH - 1 and jp == JPC // 2 - 1),
                                perf_mode=DR)

                Gacc = pp.tile([H, D], BF16, tag="Gacc")
                nc.vector.tensor_copy(Gacc[:], G[:])
                # S16 = sum over j-positions of Spsum [1, (j,h)]
                Scp = pp.tile([1, JPC * H], F32, tag="Scp")
                nc.vector.tensor_copy(Scp[:], Spsum[:])
                Ssum = pp.tile([1, H], F32, tag="Ssum")
                nc.vector.tensor_add(Ssum[:], Scp[0:1, 0:H],
                                     Scp[0:1, H:2 * H])
                for j in range(2, JPC):
                    nc.vector.tensor_add(Ssum[:], Ssum[:],
                                         Scp[0:1, j * H:(j + 1) * H])

            Sp_cm.__exit__(None, None, None)
            psG_cm.__exit__(None, None, None)
            hn_cm.__exit__(None, None, None)

            # ---- AllReduce [G | S] ----
            with tc.tile_pool(name="postps0", bufs=1, space="PSUM") as ps0:
                S16p = ps0.tile([H, 1], F32, tag="S16p")
                nc.tensor.transpose(S16p[:], Ssum[:], idn_s[0:1, 0:1])
                S16 = pp.tile([H, 1], BF16, tag="S16")
                nc.vector.tensor_copy(S16[:], S16p[:])

            arin = dram.tile([H, D + 1], BF16, tag="arin")
            arout = dram.tile([H, D + 1], BF16, tag="arout")
            nc.sync.dma_start(out=arin[:, 0:D], in_=Gacc[:])
            nc.sync.dma_start(out=arin[:, D:D + 1], in_=S16[:])
            nc.gpsimd.collective_compute(
                "AllReduce", OP.add,
                replica_groups=[list(range(ncores))],
                ins=[arin.opt()], outs=[arout.opt()])

            # ---- post: h_c_new, a0, g0 ----
            with (
                tc.tile_pool(name="postsb", bufs=1) as psb,
                tc.tile_pool(name="postps", bufs=1, space="PSUM") as ps2,
            ):
                gb16_s = psb.tile([H, D], F32, tag="gb16")
                nc.sync.dma_start(out=gb16_s[:], in_=gb16[:])
                bb16_s = psb.tile([H, D], F32, tag="bb16")
                nc.sync.dma_start(out=bb16_s[:], in_=bb16[:])
                Gar = psb.tile([H, D], BF16, tag="Gar")
                nc.sync.dma_start(out=Gar[:], in_=arout[:, 0:D])
                Sar16 = psb.tile([H, 1], BF16, tag="Sar16")
                nc.sync.dma_start(out=Sar16[:], in_=arout[:, D:D + 1])
                Sar = psb.tile([H, 1], F32, tag="Sar")
                nc.vector.tensor_copy(Sar[:], Sar16[:])
                # PRM = rowsum(G)/D  (exact identity)
                PRMt = psb.tile([H, 1], F32, tag="PRMt")
                nc.vector.reduce_sum(PRMt[:], Gar[:], axis=AX.X)
                nc.vector.tensor_scalar_mul(PRMt[:], PRMt[:], 1.0 / D)
                Gn = psb.tile([H, D], F32, tag="Gn")
                nc.vector.tensor_scalar_sub(Gn[:], Gar[:], PRMt[:, 0:1])
                nc.vector.tensor_mul(Gn[:], Gn[:], gb16_s[:])
                nc.vector.scalar_tensor_tensor(
                    Gn[:], bb16_s[:], Sar[:, 0:1], Gn[:],
                    op0=OP.mult, op1=OP.add)
                sr = psb.tile([H, 1], F32, tag="sr")
                nc.vector.reciprocal(sr[:], Sar[:])
                nc.vector.tensor_scalar_mul(Gn[:], Gn[:], sr[:, 0:1])

                tpg = ps2.tile([128, KT, H], F32, tag="tpg")
                for m in range(KT):
                    nc.tensor.transpose(
                        tpg[:, m:m + 1, :],
                        Gn[:, m * 128:(m + 1) * 128],
                        idn_s[0:H, 0:H])
                GnT = psb.tile([128, KT, H], F8, tag="GnT")
                nc.vector.tensor_copy(GnT[:], tpg[:])

                # out_center block-diag of (Gn @ Wv), scaled 2^kv
                OCp = ps2.tile([128, KT, H], F32, tag="OCp")
                for m in range(KT):
                    for k in range(KT):
                        nc.tensor.matmul(
                            OCp[:, m:m + 1, :],
                            Wv_s[:, k:k + 1, m * 128:(m + 1) * 128],
                            GnT[:, k:k + 1, :],
                            start=(k == 0), stop=(k == KT - 1))
                ocv = pp.tile([128, KT], F8, tag="ocv")
                for m in range(KT):
                    nc.vector.tensor_copy(
                        ocv[0:64, m:m + 1],
                        OCp[0:64, m:m + 1, 2 * m:2 * m + 1])
                    nc.vector.tensor_copy(
                        ocv[64:128, m:m + 1],
                        OCp[64:128, m:m + 1, 2 * m + 1:2 * m + 2])

                hcp = ps2.tile([128, KT], F32, tag="hcp")
                for m in range(KT):
                    for k in range(KT):
                        nc.tensor.matmul(
                            hcp[:, m:m + 1],
                            Wo_s[:, k:k + 1, m * 128:(m + 1) * 128],
                            ocv[:, k:k + 1],
                            start=(k == 0), stop=(k == KT - 1))
                nc.vector.scalar_tensor_tensor(
                    hcn_sb[:], hcp[:], scvo_s[:, 0:1], hcv_s[:],
                    op0=OP.mult, op1=OP.add)
                nc.sync.dma_start(out=outC[:], in_=hcn_sb[:])
                hcn8 = pp.tile([128, KT], F8, tag="hcn8")
                nc.vector.tensor_copy(hcn8[:], hcn_sb[:])

                g0p = ps2.tile([128, KT], F32, tag="g0p")
                for m in range(KT):
                    for k in range(KT):
                        nc.tensor.matmul(
                            g0p[:, m:m + 1],
                            Wgb_s[:, k:k + 1, m * 128:(m + 1) * 128],
                            hcn8[:, k:k + 1],
                            start=(k == 0), stop=(k == KT - 1))
                nc.vector.scalar_tensor_tensor(
                    g0_s[:], g0p[:], scb_s[:, 0:1], bgv_s[:],
                    op0=OP.mult, op1=OP.add)
                a0p = ps2.tile([128, KT], F32, tag="a0p")
                for m in range(KT):
                    for k in range(KT):
                        nc.tensor.matmul(
                            a0p[:, m:m + 1],
                            W1b_s[:, k:k + 1, m * 128:(m + 1) * 128],
                            hcn8[:, k:k + 1],
                            start=(k == 0), stop=(k == KT - 1))
                nc.vector.scalar_tensor_tensor(
                    a0_s[:], a0p[:], scb_s[:, 0:1], b1v_s[:],
                    op0=OP.mult, op1=OP.add)

            # ========================= PASS 2 =============================
            # Two sweeps per group of 4 chunks: all-Silu then all-Sigmoid,
            # so the ACT engine switches table sets ~4x instead of ~150x.
            # Fences: the Silu bias (a0_f) / Sigmoid bias (g0_f) of each
            # sweep carry a data dependency on the previous sweep's last
            # activation output, so the scheduler cannot interleave sets.
            GRP = 4
            a0_cur = a0_s
            g0_last_gs = None
            with (
                tc.tile_pool(name="p2sbA", bufs=2) as sbA,
                tc.tile_pool(name="p2sbB", bufs=2) as sbB,
                tc.tile_pool(name="p2bg", bufs=1) as sbg,
                tc.tile_pool(name="p2st", bufs=3) as sb4,
                tc.tile_pool(name="p2fence", bufs=4) as sbf,
                tc.tile_pool(name="p2ps", bufs=2, space="PSUM") as ps3,
            ):
                for g in range(NCH // GRP):
                    # ---- sweep A: Za matmuls + Silu -> B8 ----
                    if g0_last_gs is not None:
                        fa = sbf.tile([128, 1], F32, tag="fa")
                        nc.vector.tensor_copy(fa[:], g0_last_gs)
                        a0_cur = sbf.tile([128, KT], F32, tag="a0f")
                        nc.vector.tensor_scalar(
                            out=a0_cur[:], in0=a0_s[:],
                            scalar1=fa[:, 0:1], scalar2=fa[:, 0:1],
                            op0=OP.add, op1=OP.subtract)
                    B8 = sbg.tile([128, GRP * KT, CH], F8, tag="B8")
                    for ci in range(GRP):
                        c = g * GRP + ci
                        c0 = c * CH
                        hTcA = sbA.tile([128, KT, CH], F8, tag="hTcA")
                        for k in range(KT):
                            nc.sync.dma_start(
                                out=hTcA[:, k:k + 1, :],
                                in_=hT8[k * 128:(k + 1) * 128, c0:c0 + CH])
                        for m in range(KT):
                            for half in range(2):
                                A = ps3.tile([128, 512], F32, tag="A")
                                for kp in range(KT // 2):
                                    nc.tensor.matmul(
                                        A[:],
                                        W1t_s[:, 2 * kp:2 * kp + 2,
                                              m * 128:(m + 1) * 128],
                                        hTcA[:, 2 * kp:2 * kp + 2,
                                             half * 512:(half + 1) * 512],
                                        start=(kp == 0),
                                        stop=(kp == KT // 2 - 1),
                                        perf_mode=DR)
                                nc.scalar.activation(
                                    B8[:, ci * KT + m:ci * KT + m + 1,
                                       half * 512:(half + 1) * 512],
                                    A[:], AF.Silu,
                                    bias=a0_cur[:, m:m + 1],
                                    scale=sck2_s[:, 0:1])
                    # ---- sweep B: gate + fuse + residual + store ----
                    fg = sbf.tile([128, 1], F32, tag="fg")
                    nc.vector.tensor_copy(
                        fg[:], B8[:, GRP * KT - 1:GRP * KT, CH - 1:CH])
                    g0_cur = sbf.tile([128, KT], F32, tag="g0f")
                    nc.vector.tensor_scalar(
                        out=g0_cur[:], in0=g0_s[:],
                        scalar1=fg[:, 0:1], scalar2=fg[:, 0:1],
                        op0=OP.add, op1=OP.subtract)
                    for ci in range(GRP):
                        c = g * GRP + ci
                        c0 = c * CH
                        hTcB = sbB.tile([128, KT, CH], F8, tag="hTcB")
                        for k in range(KT):
                            nc.sync.dma_start(
                                out=hTcB[:, k:k + 1, :],
                                in_=hT8[k * 128:(k + 1) * 128, c0:c0 + CH])
                        hTbc = sbB.tile([128, KT, CH], BF16, tag="hTbc")
                        for k in range(KT):
                            nc.sync.dma_start(
                                out=hTbc[:, k:k + 1, :],
                                in_=hTb[k * 128:(k + 1) * 128, c0:c0 + CH])
                        for m in range(KT):
                            for half in range(2):
                                Gt = ps3.tile([128, 512], F32, tag="Gt")
                                for kp in range(KT // 2):
                                    nc.tensor.matmul(
                                        Gt[:],
                                        Wgt_s[:, 2 * kp:2 * kp + 2,
                                              m * 128:(m + 1) * 128],
                                        hTcB[:, 2 * kp:2 * kp + 2,
                                             half * 512:(half + 1) * 512],
                                        start=(kp == 0),
                                        stop=(kp == KT // 2 - 1),
                                        perf_mode=DR)
                                gs = sb4.tile([128, 512], BF16, tag="gs")
                                nc.scalar.activation(
                                    gs[:], Gt[:], AF.Sigmoid,
                                    bias=g0_cur[:, m:m + 1],
                                    scale=sck2_s[:, 0:1])
                                Cp = ps3.tile([128, 512], F32, tag="Cp")
                                for kp in range(KT // 2):
                                    nc.tensor.matmul(
                                        Cp[:],
                                        W2h_s[:, 2 * kp:2 * kp + 2,
                                              m * 128:(m + 1) * 128],
                                        B8[:, ci * KT + 2 * kp:
                                           ci * KT + 2 * kp + 2,
                                           half * 512:(half + 1) * 512],
                                        start=(kp == 0),
                                        stop=(kp == KT // 2 - 1),
                                        perf_mode=DR)
                                u = sb4.tile([128, 512], F32, tag="u")
                                nc.vector.scalar_tensor_tensor(
                                    u[:], Cp[:], b2v_s[:, m:m + 1], gs[:],
                                    op0=OP.add, op1=OP.mult)
                                ot = sb4.tile([128, 512], BF16, tag="ot")
                                nc.vector.scalar_tensor_tensor(
                                    ot[:], u[:], sck2_s[:, 0:1],
                                    hTbc[:, m:m + 1,
                                         half * 512:(half + 1) * 512],
                                    op0=OP.mult, op1=OP.add)
                                nc.sync.dma_start(
                                    out=outT[m * 128:(m + 1) * 128,
                                             c0 + half * 512:
                                             c0 + (half + 1) * 512],
                                    in_=ot[:])
                                if ci == GRP - 1 and m == KT - 1 and half == 1:
                                    g0_last_gs = gs[:, 0:1]
    nc.compile()
    return nc


def _get_nc():
    if "nc" not in _CACHE:
        _CACHE["nc"] = _build()
    return _CACHE["nc"]


def _exp_scale(maxval, target=96.0):
    if maxval <= 0:
        return 0
    return int(np.floor(np.log2(target / maxval)))


def kernel(h, center_idx, rbf_ic, seqsep_ic, nbr_idx, local_bias,
           gamma_c, beta_c, gamma_a, beta_a,
           Wq, Wk, Wv, Wo, Wb, W1, b1, W2, b2, Wg, bg):
    global LAST_RESULTS
    f = np.float32
    f8 = ml_dtypes.float8_e4m3
    bf = ml_dtypes.bfloat16
    h = np.asarray(h, f)
    c = int(center_idx)
    rbf_ic = np.asarray(rbf_ic, f)
    seqsep_ic = np.asarray(seqsep_ic, f)
    nbr_idx = np.asarray(nbr_idx)
    local_bias = np.asarray(local_bias, f)
    gamma_c = np.asarray(gamma_c, np.float64)
    beta_c = np.asarray(beta_c, np.float64)
    gamma_a = np.asarray(gamma_a, np.float64)
    beta_a = np.asarray(beta_a, np.float64)
    Wq = np.asarray(Wq, f); Wk = np.asarray(Wk, f); Wv = np.asarray(Wv, f)
    Wo = np.asarray(Wo, f); Wb = np.asarray(Wb, f)
    W1 = np.asarray(W1, f); b1 = np.asarray(b1, f)
    W2 = np.asarray(W2, f); b2 = np.asarray(b2, f)
    Wg = np.asarray(Wg, f); bg = np.asarray(bg, f)

    # ---- host algebra (tiny, no big matmuls) ----
    hc = h[c].astype(np.float64)
    hcl = (hc - hc.mean()) / np.sqrt(hc.var() + EPS) * gamma_c + beta_c
    q = (hcl @ Wq.astype(np.float64)).reshape(H, HD)
    Qm = np.zeros((D, H), np.float64)
    for hh in range(H):
        Qm[hh * HD:(hh + 1) * HD, hh] = q[hh] / np.sqrt(HD)
    Wk1 = Wk.astype(np.float64) @ Qm                    # (D, 16)
    Wkp = (Wk1 * gamma_a[:, None]).astype(f)            # (D, 16)
    cg = Wkp.sum(0)                                     # (16,)

    full_bias = np.zeros((N, local_bias.shape[1]), f)
    full_bias[nbr_idx] = local_bias
    bias_featT = np.ascontiguousarray(
        np.concatenate([rbf_ic, seqsep_ic, full_bias], axis=1).T)  # (128, N)

    # scale exponents
    k1 = _exp_scale(np.abs(Wkp).max())
    k2 = _exp_scale(max(np.abs(W1[:D]).max(), np.abs(Wg[:D]).max(),
                        np.abs(RES * W2).max()))
    kv = _exp_scale(np.abs(Wv).max())
    ko = _exp_scale(np.abs(Wo).max())
    kb = _exp_scale(max(np.abs(W1[D:]).max(), np.abs(Wg[D:]).max()))

    def pack3(W, s):
        # (D, Dout) -> [128, KT, Dout] scaled fp8
        return np.ascontiguousarray(
            (W * s).reshape(KT, 128, -1).transpose(1, 0, 2)).astype(f8)

    gamma_a32 = gamma_a.astype(f)
    beta_a32 = beta_a.astype(f)
    shared = {
        "Wkp8": pack3(Wkp, 2.0 ** k1),
        "Wb8": Wb.astype(f8),
        "W1t8": pack3(W1[:D], 2.0 ** k2),
        "Wgt8": pack3(Wg[:D], 2.0 ** k2),
        "W2h8": pack3(RES * W2, 2.0 ** k2),
        "Wv8": pack3(Wv, 2.0 ** kv),
        "Wo8": pack3(Wo, 2.0 ** ko),
        "W1b8": pack3(W1[D:], 2.0 ** kb),
        "Wgb8": pack3(Wg[D:], 2.0 ** kb),
        "idn": np.eye(128, dtype=f),
        "ones128b": np.ones((128, 1), bf),
        "cgt": np.ascontiguousarray(np.broadcast_to(-cg, (128, H))),
        "gb16": np.ascontiguousarray(np.broadcast_to(gamma_a32, (H, D))),
        "bb16": np.ascontiguousarray(np.broadcast_to(beta_a32, (H, D))),
        "hcv": np.ascontiguousarray(h[c].reshape(KT, 128).T),
        "b1v": np.ascontiguousarray(b1.reshape(KT, 128).T),
        "bgv": np.ascontiguousarray(bg.reshape(KT, 128).T),
        "b2v2": np.ascontiguousarray(
            (RES * b2 * 2.0 ** k2).reshape(KT, 128).T),
        "epsv": np.full((128, 1), EPS, f),
        "sck1": np.full((128, 1), 2.0 ** -k1, f),
        "sck2": np.full((128, 1), 2.0 ** -k2, f),
        "scvo": np.full((128, 1), RES * 2.0 ** -(kv + ko), f),
        "scb": np.full((128, 1), 2.0 ** -kb, f),
    }
    in_maps = []
    for i in range(NCORES):
        r0 = i * NS
        sh = h[r0:r0 + NS]
        shT = np.ascontiguousarray(sh.T)
        m = dict(shared)
        m["hN8"] = sh.astype(f8)
        m["hT8"] = shT.astype(f8)
        m["hTb"] = shT.astype(bf)
        m["bT8"] = np.ascontiguousarray(
            bias_featT[:, r0:r0 + NS]).astype(f8)
        in_maps.append(m)

    nc = _get_nc()
    trace = bool(int(os.environ.get("KERNEL_TRACE", "0")))
    res = run_bass_kernel_spmd(nc, in_maps, core_ids=list(range(NCORES)),
                               trace=trace)
    LAST_RESULTS = res

    out = np.empty((N, D), f)
    for i in range(NCORES):
        out[i * NS:(i + 1) * NS] = res.results[i]["outT"].T.astype(f)
    hcn = res.results[0]["outC"].T.reshape(D)           # [m,p] -> flat
    out[c] = hcn
    return out
